# revision 1
# baseline (speedup 1.0000x reference)
"""Trainium2 Bass kernel for: x + s -> LayerNorm(W) -> 2x2x2 avgpool -> exact GELU.

Input  x: (32, 32, 16, 32, 64) f32, sum_weight (1,), gamma (64,), beta (64,)
Output:   (32, 32, 8, 16, 32) f32

Math notes:
  v = x + s;  LN over last dim W: mean/var are shift-equivariant/invariant, so
  (v - mean_v) = (x - mean_x) and var_v = var_x  ==> sum_weight cancels exactly.
  ln = (x - mu) * rho * gamma + beta,  rho = rsqrt(var + eps)
  pooled[q, w'] = (1/8) [ S - gw[w'] * M4 + 4*(beta_e+beta_o)[w'] ]
    S   = sum_{r in quad} rho_r * (ga*x[r,2w'] + go*x[r,2w'+1])  (ga/go = even/odd gamma)
    M4  = sum_{r in quad} mu_r * rho_r,   gw = ga + go
  out = 0.5 * p * (1 + erf(p/sqrt(2))) = Gelu(p)

Layout: data-parallel over batch N (4 per core x 8 cores). On each core,
partition dim = the 128 (n, c) pairs; free dim = (d, h, w). All LN rows and all
pooling directions live along the free dimension, so the kernel is pure
DVE/ACT/GPSIMD elementwise + bn_stats work with fully contiguous DMA.
"""

import numpy as np

import concourse.bacc as bacc
import concourse.bass as bass
import concourse.tile as tile
from concourse import mybir
from concourse.bass_utils import run_bass_kernel_spmd

P = 128
N, C, D, H, W = 32, 32, 16, 32, 64
NCORES = 8
NPER = N // NCORES  # batches per core
EPS = 1e-5
F32 = mybir.dt.float32

# rows (d,h) per chunk = one d-pair * H = 64 rows of W=64 -> 4096 f32/partition
CHUNK_ELEMS = 2 * H * W  # 4096
NCHUNK = D // 2  # 8

# Fraction of the xr (x * rstd) pass done on GPSIMD (rest on DVE); rows of 64.
XR_GP_ROWS = 64  # all 64 rows on gpsimd
# d-pool split: columns (of 2048) handled by gpsimd
DPOOL_GP_COLS = 0
# h-pool on gpsimd?
HPOOL_GP = True


def _kernel_body(
    ctx, tc: tile.TileContext, out_ap: bass.AP, xs: bass.AP, cons: bass.AP
):
    nc = tc.nc

    singles = ctx.enter_context(tc.tile_pool(name="singles", bufs=1))
    xpool = ctx.enter_context(tc.tile_pool(name="xpool", bufs=3))
    sqpool = ctx.enter_context(tc.tile_pool(name="sqpool", bufs=2))
    workbig = ctx.enter_context(tc.tile_pool(name="workbig", bufs=2))
    work = ctx.enter_context(tc.tile_pool(name="work", bufs=2))
    small = ctx.enter_context(tc.tile_pool(name="small", bufs=3))

    # constants, broadcast to all partitions
    ga_t = singles.tile([P, 32], F32)
    go_t = singles.tile([P, 32], F32)
    gw_t = singles.tile([P, 32], F32)
    bw_t = singles.tile([P, 32], F32)
    for r, t in enumerate((ga_t, go_t, gw_t, bw_t)):
        nc.sync.dma_start(out=t[:], in_=cons[r : r + 1, :].to_broadcast((P, 32)))
    eps_t = singles.tile([P, 1], F32)
    nc.vector.memset(eps_t[:], EPS)
    inv64_t = singles.tile([P, 1], F32)
    nc.vector.memset(inv64_t[:], 1.0 / W)

    xsf = xs.rearrange("p d h w -> p (d h w)")
    outf = out_ap.rearrange("p d h w -> p d (h w)")

    for k in range(NCHUNK):
        xc = xpool.tile([P, CHUNK_ELEMS], F32, tag="xc")
        nc.sync.dma_start(
            out=xc[:], in_=xsf[:, k * CHUNK_ELEMS : (k + 1) * CHUNK_ELEMS]
        )

        # --- per-row stats: sum and sum-of-squares reductions over W ---
        xc3v = xc[:].rearrange("p (r w) -> p r w", w=W)
        sq = sqpool.tile([P, CHUNK_ELEMS], F32, tag="sq")
        nc.scalar.activation(sq[:], xc[:], mybir.ActivationFunctionType.Square)
        r1 = small.tile([P, 64], F32, tag="r1")
        nc.vector.tensor_reduce(
            out=r1[:], in_=xc3v, axis=mybir.AxisListType.X, op=mybir.AluOpType.add
        )
        r2 = small.tile([P, 64], F32, tag="r2")
        nc.vector.tensor_reduce(
            out=r2[:],
            in_=sq[:].rearrange("p (r w) -> p r w", w=W),
            axis=mybir.AxisListType.X,
            op=mybir.AluOpType.add,
        )
        # msq = r1^2; v64 = r2 - r1^2/64 (= 64*var); rstd = 1/sqrt(v64/64+eps)
        # Stats smalls go to GPSIMD: only the (port-safe) reduces/reciprocal
        # stay on DVE, so the GPSIMD xr window doesn't stall DVE TT ops.
        msq = small.tile([P, 64], F32, tag="msq")
        nc.gpsimd.tensor_mul(msq[:], r1[:], r1[:])
        m64 = small.tile([P, 64], F32, tag="m64")
        nc.gpsimd.tensor_mul(m64[:], msq[:], inv64_t[:].to_broadcast((P, 64)))
        v64 = small.tile([P, 64], F32, tag="v64")
        nc.gpsimd.tensor_sub(v64[:], r2[:], m64[:])
        rstd = small.tile([P, 64], F32, tag="rstd")
        nc.scalar.activation(
            rstd[:],
            v64[:],
            mybir.ActivationFunctionType.Sqrt,
            bias=eps_t[:],
            scale=1.0 / W,
        )
        nc.vector.reciprocal(out=rstd[:], in_=rstd[:])
        # mrs = 64 * mu * rho = r1 * rstd  (the 1/64 is folded into the gw
        # constant on the host side)
        mrs = small.tile([P, 64], F32, tag="mrs")
        nc.gpsimd.tensor_mul(mrs[:], r1[:], rstd[:])

        # --- xr = x * rstd (broadcast rstd over each row of 64) ---
        xr = workbig.tile([P, CHUNK_ELEMS], F32, tag="xr")
        xc3 = xc[:].rearrange("p (r w) -> p r w", w=W)
        xr3 = xr[:].rearrange("p (r w) -> p r w", w=W)
        g = XR_GP_ROWS
        if g > 0:
            nc.gpsimd.tensor_tensor(
                out=xr3[:, :g, :],
                in0=xc3[:, :g, :],
                in1=rstd[:, :g].unsqueeze(2).to_broadcast((P, g, W)),
                op=mybir.AluOpType.mult,
            )
        if g < 64:
            nc.vector.tensor_tensor(
                out=xr3[:, g:, :],
                in0=xc3[:, g:, :],
                in1=rstd[:, g:].unsqueeze(2).to_broadcast((P, 64 - g, W)),
                op=mybir.AluOpType.mult,
            )

        # --- d-pool: rows (dd, h) -> sum over dd ---
        xd = workbig.tile([P, H * W], F32, tag="xd")  # [P, 2048]
        xr_d = xr[:].rearrange("p (d r) -> p d r", d=2)
        c = DPOOL_GP_COLS
        if c > 0:
            nc.gpsimd.tensor_tensor(
                out=xd[:, :c],
                in0=xr_d[:, 0, :c],
                in1=xr_d[:, 1, :c],
                op=mybir.AluOpType.add,
            )
        if c < H * W:
            nc.vector.tensor_tensor(
                out=xd[:, c:],
                in0=xr_d[:, 0, c:],
                in1=xr_d[:, 1, c:],
                op=mybir.AluOpType.add,
            )

        # --- h-pool: [P, 32, 64] -> [P, 16, 64] ---
        xh = work.tile([P, 16, W], F32, tag="xh")
        xd3 = xd[:].rearrange("p (h t w) -> p h t w", t=2, w=W)
        heng = nc.gpsimd if HPOOL_GP else nc.vector
        heng.tensor_tensor(
            out=xh[:], in0=xd3[:, :, 0, :], in1=xd3[:, :, 1, :], op=mybir.AluOpType.add
        )

        # --- gamma combine: s = ga*xh_even + go*xh_odd  -> [P, 16, 32] ---
        xh4 = xh[:].rearrange("p h (v t) -> p h v t", t=2)
        t1 = work.tile([P, 16, 32], F32, tag="t1")
        nc.vector.tensor_tensor(
            out=t1[:],
            in0=xh4[:, :, :, 0],
            in1=ga_t[:].unsqueeze(1).to_broadcast((P, 16, 32)),
            op=mybir.AluOpType.mult,
        )
        t2 = work.tile([P, 16, 32], F32, tag="t2")
        nc.vector.tensor_tensor(
            out=t2[:],
            in0=xh4[:, :, :, 1],
            in1=go_t[:].unsqueeze(1).to_broadcast((P, 16, 32)),
            op=mybir.AluOpType.mult,
        )
        s = work.tile([P, 16, 32], F32, tag="s")
        nc.vector.tensor_add(s[:], t1[:], t2[:])

        # --- correction: M4 per quad, corr = gw * M4 ---
        m1 = small.tile([P, 32], F32, tag="m1")
        mrs_d = mrs[:].rearrange("p (d h) -> p d h", d=2)
        nc.gpsimd.tensor_add(m1[:], mrs_d[:, 0, :], mrs_d[:, 1, :])
        mq = small.tile([P, 16], F32, tag="mq")
        m1p = m1[:].rearrange("p (h t) -> p h t", t=2)
        nc.gpsimd.tensor_add(mq[:], m1p[:, :, 0], m1p[:, :, 1])

        corr = work.tile([P, 16, 32], F32, tag="corr")
        nc.vector.tensor_tensor(
            out=corr[:],
            in0=mq[:].unsqueeze(2).to_broadcast((P, 16, 32)),
            in1=gw_t[:].unsqueeze(1).to_broadcast((P, 16, 32)),
            op=mybir.AluOpType.mult,
        )
        pre = work.tile([P, 16, 32], F32, tag="pre")
        nc.vector.tensor_sub(pre[:], s[:], corr[:])
        pre2 = work.tile([P, 16, 32], F32, tag="pre2")
        nc.vector.tensor_tensor(
            out=pre2[:],
            in0=pre[:],
            in1=bw_t[:].unsqueeze(1).to_broadcast((P, 16, 32)),
            op=mybir.AluOpType.add,
        )

        # --- GELU(pre2 / 8) ---
        res = work.tile([P, 16 * 32], F32, tag="res")
        nc.scalar.activation(
            res[:],
            pre2[:].rearrange("p a b -> p (a b)"),
            mybir.ActivationFunctionType.Gelu,
            scale=0.125,
        )
        nc.sync.dma_start(out=outf[:, k, :], in_=res[:])


_CACHE: dict = {}


def _get_compiled():
    if "nc" not in _CACHE:
        nc = bacc.Bacc("TRN2", target_bir_lowering=False, debug=False)
        xs = nc.dram_tensor("xs", [P, D, H, W], F32, kind="ExternalInput").ap()
        cons = nc.dram_tensor("cons", [4, 32], F32, kind="ExternalInput").ap()
        out = nc.dram_tensor(
            "out", [P, D // 2, H // 2, W // 2], F32, kind="ExternalOutput"
        ).ap()
        from contextlib import ExitStack

        with tile.TileContext(nc) as tc, ExitStack() as ctx:
            _kernel_body(ctx, tc, out, xs, cons)
        nc.compile()
        _CACHE["nc"] = nc
    return _CACHE["nc"]


def _make_cons(gamma: np.ndarray, beta: np.ndarray) -> np.ndarray:
    ga = gamma[0::2].astype(np.float32)
    go = gamma[1::2].astype(np.float32)
    gw = (ga + go) / 64.0  # mrs carries an extra factor of 64
    bw = 4.0 * (beta[0::2] + beta[1::2]).astype(np.float32)
    return np.stack([ga, go, gw, bw]).astype(np.float32)


def kernel(x, sum_weight, gamma, beta, trace=False):
    del sum_weight  # cancels exactly in LayerNorm (shift invariance)
    nc = _get_compiled()
    x = np.ascontiguousarray(np.asarray(x), dtype=np.float32)
    cons = _make_cons(np.asarray(gamma), np.asarray(beta))
    in_maps = []
    for core in range(NCORES):
        shard = x[core * NPER : (core + 1) * NPER].reshape(P, D, H, W)
        in_maps.append({"xs": shard, "cons": cons})
    res = run_bass_kernel_spmd(nc, in_maps, core_ids=list(range(NCORES)), trace=trace)
    out = np.concatenate(
        [
            res.results[i]["out"].reshape(NPER, C, D // 2, H // 2, W // 2)
            for i in range(NCORES)
        ],
        axis=0,
    )
    if trace:
        return out, res
    return out


if __name__ == "__main__":
    rng = np.random.default_rng(0)
    x = rng.standard_normal((N, C, D, H, W), dtype=np.float32)
    sw = rng.standard_normal((1,)).astype(np.float32)
    gamma = rng.random((W,), dtype=np.float32)
    beta = rng.standard_normal((W,)).astype(np.float32)
    y = kernel(x, sw, gamma, beta)
    print(y.shape, y.dtype)

